# revision 33
# baseline (speedup 1.0000x reference)
"""Trainium2 Bass kernel for BoltzmannMoE (top-2 of 8 experts, N=8192, D=1024, H=4096, O=1024).

Strategy (expert-parallel across 8 NeuronCores):
  - Host: gate (softmax -> top-2 -> renormalize) in numpy fp32, gather each
    expert's tokens, run one expert per core, weighted scatter-add on host.
    Device capacity is the mean expert load (2048 = 4x512 uniform pieces);
    the ~1% of assignments past capacity fall back to host fp32.
  - Device (per core, SPMD), fp32 PSUM accumulation throughout; tokens
    processed in 512-column pieces. Per piece:
      mm1: h = relu(W1^T @ xg + b1)   (W1 tiles stationary, streamed per
           piece). 6 of 8 k-subtiles run in bf16; the last 2 run as a single
           fp8-e4m3 DoubleRow matmul (256-deep contraction per column
           stream, 2x rate). Everything accumulates at 64x scale -- W1*64
           fits fp8 range, relu commutes with the scale, and W2/64 undoes it,
           all exact power-of-2 folds. Measured end-to-end rel err 1.85e-2
           (gate 2e-2), bit-reproducible across runs.
      mm2: y[tok_tile] = ht^T @ W2    (token tiles stationary, W2 moving --
                                       W2 SBUF-resident, loaded once), bf16.
    A burst of junk matmuls on iota data at launch releases the PE HAM
    clock-gate (starts throttled ~2x) while the input DMAs spin up.
"""

import numpy as np
import ml_dtypes

import concourse.bass as bass
import concourse.mybir as mybir
import concourse.tile as tile
from concourse import bacc
from concourse.bass_utils import run_bass_kernel_spmd

P = 128
D, H, O, E, KTOP = 1024, 4096, 1024, 8, 2
TEMP = 2.718281828459045
NCORES = 8

DK = D // P    # 8  k-subtiles for mm1
BK = 6         # mm1 k-subtiles computed in bf16
FK = DK - BK   # 2  mm1 k-subtiles computed in fp8-e4m3 via one DoubleRow matmul
SCALE = 64.0   # power-of-2 fold: W1*64 (fp8/bf16 range), W2/64 undoes it
HK = H // P    # 32 k-subtiles for mm2
HT = H // P    # 32 h output tiles (mm1 M dim)
OHALF = 2      # mm2 output halves (512 cols each)
W1_POOL = 6    # w1 pool depth
W1_PRE = 2     # w1 tiles preloaded ahead of the first xg piece

BF16 = mybir.dt.bfloat16
F8 = mybir.dt.float8e4

LAST_RESULTS = None  # BassKernelResults of the most recent device run (for test harness)


def _pieces(C):
    """C is always a multiple of 512: uniform full-width pieces. Tokens
    beyond the device capacity are handled on the host (capacity-factor
    overflow), so no short-tail code path exists on device."""
    assert C % 512 == 0
    return [{"pack": i * 512, "s0": i * 512, "sz": 512} for i in range(C // 512)]


def _ttiles(sz):
    """Token-tiles (stationary blocks) within a piece."""
    out = []
    off = 0
    while off < sz:
        t = min(P, sz - off)
        out.append((off, t))
        off += t
    return out


def _build_program(C):
    nc = bacc.Bacc("TRN2", target_bir_lowering=False, debug=False)

    pieces = _pieces(C)
    TT = C // P  # token-tiles

    # xgT packed piece-major: piece p occupies flat cols
    # [BK*pack, BK*(pack+sz)) laid out as [BK, sz] (k-major within piece);
    # xg8 holds the last FK k-subtiles in fp8 as DoubleRow pair slots.
    xgT = nc.dram_tensor("xgT", (P, BK * C), BF16, kind="ExternalInput")
    xg8 = nc.dram_tensor("xg8", (P, FK * C), F8, kind="ExternalInput")
    w1 = nc.dram_tensor("w1", (HT, P, BK, P), BF16, kind="ExternalInput")
    w18 = nc.dram_tensor("w18", (HT, P, FK, P), F8, kind="ExternalInput")
    w2 = nc.dram_tensor("w2", (P, HK, O), BF16, kind="ExternalInput")
    b1 = nc.dram_tensor("b1", (P, HT), mybir.dt.float32, kind="ExternalInput")
    yT = nc.dram_tensor("yT", (TT, P, O), mybir.dt.float32, kind="ExternalOutput")

    with tile.TileContext(nc) as tc:
        with (
            tc.tile_pool(name="const", bufs=2) as const,
            tc.tile_pool(name="w1p", bufs=W1_POOL) as w1_pool,
            tc.tile_pool(name="w18p", bufs=W1_POOL) as w18_pool,
            tc.tile_pool(name="w2r", bufs=1) as w2_pool,
            tc.tile_pool(name="xg", bufs=3) as xg_pool,
            tc.tile_pool(name="xg8", bufs=3) as xg8_pool,
            tc.tile_pool(name="ht", bufs=2) as ht_pool,
            tc.tile_pool(name="yst", bufs=4) as yst_pool,
            tc.tile_pool(name="psj", bufs=1, space="PSUM") as psj,
            tc.tile_pool(name="psa", bufs=3, space="PSUM") as psa,
            tc.tile_pool(name="psb", bufs=4, space="PSUM") as psb,
        ):
            b1_sb = const.tile([P, HT], mybir.dt.float32)
            nc.sync.dma_start(b1_sb[:], b1.ap())

            def dma_xg_piece(pi):
                pack, sz = pieces[pi]["pack"], pieces[pi]["sz"]
                t = xg_pool.tile([P, BK, 512], BF16, name="xg_t")
                for k in range(BK):
                    nc.sync.dma_start(
                        t[:, k, :sz],
                        xgT.ap()[:, BK * pack + k * sz : BK * pack + (k + 1) * sz],
                    )
                t8 = xg8_pool.tile([P, FK, 512], F8, name="xg8_t")
                for j in range(FK):
                    nc.sync.dma_start(
                        t8[:, j, :sz],
                        xg8.ap()[:, FK * pack + j * sz : FK * pack + (j + 1) * sz],
                    )
                return t, t8

            # HAM warmup: the PE clock-gate starts throttled and releases
            # after ~4us of sustained activity. Burn junk matmuls on a
            # memset tile (no DMA dependency, starts immediately) while the
            # input DMAs spin up, so real matmuls run at full clock.
            junk = const.tile([P, P], BF16)
            nc.gpsimd.iota(
                junk[:],
                pattern=[[1, P]],
                base=1,
                channel_multiplier=7,
                allow_small_or_imprecise_dtypes=True,
            )
            ps_junk = psj.tile([P, 512], mybir.dt.float32, name="ps_junk")
            NJUNK = 40
            for j in range(NJUNK):
                nc.tensor.matmul(
                    ps_junk[:, :P],
                    junk[:],
                    junk[:],
                    start=(j == 0),
                    stop=(j == NJUNK - 1),
                )

            # piece-0 critical path: w1 tile 0, first xg k-slice, w1 tile 1,
            # rest of xg piece 0
            w1_pre, w18_pre = [], []

            def dma_w1(ht):
                t = w1_pool.tile([P, BK, P], BF16, name="w1_t")
                nc.sync.dma_start(t[:], w1.ap()[ht])
                t8 = w18_pool.tile([P, FK, P], F8, name="w18_t")
                nc.sync.dma_start(t8[:], w18.ap()[ht])
                return t, t8

            t, t8 = dma_w1(0)
            w1_pre.append(t)
            w18_pre.append(t8)
            pk0 = pieces[0]["pack"]
            xg_next = xg_pool.tile([P, BK, 512], BF16, name="xg_t")
            nc.sync.dma_start(xg_next[:, 0, :], xgT.ap()[:, BK * pk0 : BK * pk0 + 512])
            t, t8 = dma_w1(1)
            w1_pre.append(t)
            w18_pre.append(t8)
            for k in range(1, BK):
                nc.sync.dma_start(
                    xg_next[:, k, :],
                    xgT.ap()[:, BK * pk0 + k * 512 : BK * pk0 + (k + 1) * 512],
                )
            xg8_next = xg8_pool.tile([P, FK, 512], F8, name="xg8_t")
            for j in range(FK):
                nc.sync.dma_start(
                    xg8_next[:, j, :],
                    xg8.ap()[:, FK * pk0 + j * 512 : FK * pk0 + (j + 1) * 512],
                )

            w2_sb = w2_pool.tile([P, HK, O], BF16, name="w2_sb")

            for pi, pc in enumerate(pieces):
                s0, sz = pc["s0"], pc["sz"]
                xg_t, xg8_t = xg_next, xg8_next

                # ---- mm1: ht = relu(W1^T @ xg + b1), 64x-scaled psum ----
                ht_t = ht_pool.tile([P, HK, 512], BF16, name="ht_t")
                for ht in range(HT):
                    if pi == 0 and ht < W1_PRE:
                        w1_t, w18_t = w1_pre[ht], w18_pre[ht]
                    else:
                        w1_t, w18_t = dma_w1(ht)
                    ps = psa.tile([P, 512], mybir.dt.float32, name="ps_a")
                    for k in range(BK):
                        nc.tensor.matmul(
                            ps[:, :sz],
                            w1_t[:, k, :],
                            xg_t[:, k, :sz],
                            start=(k == 0),
                            stop=False,
                        )
                    # fp8 DoubleRow: both remaining k-subtiles in one matmul
                    nc.tensor.matmul(
                        ps[:, :sz],
                        w18_t[:],
                        xg8_t[:, :, :sz],
                        start=False,
                        stop=True,
                        perf_mode=mybir.MatmulPerfMode.DoubleRow,
                    )
                    nc.scalar.activation(
                        ht_t[:, ht, :sz],
                        ps[:, :sz],
                        mybir.ActivationFunctionType.Relu,
                        bias=b1_sb[:, ht : ht + 1],
                    )

                # next xg piece, then (once) the resident W2
                if pi + 1 < len(pieces):
                    xg_next, xg8_next = dma_xg_piece(pi + 1)
                if pi == 0:
                    for k in range(HK):
                        nc.sync.dma_start(w2_sb[:, k], w2.ap()[:, k])

                # ---- mm2: y[tok_tile] = ht^T @ W2 (tokens stationary) ----
                tt_base = s0 // P
                for ti, (toff, tw) in enumerate(_ttiles(sz)):
                    for oh in range(OHALF):
                        ps = psb.tile([P, 512], mybir.dt.float32, name="ps_b")
                        for k in range(HK):
                            nc.tensor.matmul(
                                ps[:tw, :],
                                ht_t[:, k, toff : toff + tw],
                                w2_sb[:, k, oh * 512 : (oh + 1) * 512],
                                start=(k == 0),
                                stop=(k == HK - 1),
                            )
                        st = yst_pool.tile([P, 512], mybir.dt.float32, name="y_st")
                        nc.vector.tensor_copy(st[:tw, :], ps[:tw, :])
                        nc.sync.dma_start(
                            yT.ap()[tt_base + ti][:tw, oh * 512 : (oh + 1) * 512],
                            st[:tw, :],
                        )

    nc.compile()
    return nc


def _host_gate(x, Wg, bg):
    """Replicates reference gating in fp32: softmax(scores/T) -> top-2 -> renorm."""
    scores = (x @ Wg + bg) / np.float32(TEMP)
    m = scores.max(axis=-1, keepdims=True)
    un = np.exp(scores - m)
    probs = un / un.sum(-1, keepdims=True)
    order = np.argsort(-probs, axis=1, kind="stable")[:, :KTOP]
    vals = np.take_along_axis(probs, order, axis=1)
    w = np.zeros_like(probs)
    np.put_along_axis(w, order, vals, axis=1)
    w = w / (w.sum(-1, keepdims=True) + np.float32(1e-8))
    return w


def kernel(x, Wg, bg, W1, b1, W2, b2):
    global LAST_RESULTS
    x = np.ascontiguousarray(np.asarray(x, dtype=np.float32))
    Wg = np.asarray(Wg, dtype=np.float32)
    bg = np.asarray(bg, dtype=np.float32)
    W1 = np.asarray(W1, dtype=np.float32)
    b1 = np.asarray(b1, dtype=np.float32)
    W2 = np.asarray(W2, dtype=np.float32)
    b2 = np.asarray(b2, dtype=np.float32)
    N = x.shape[0]

    w = _host_gate(x, Wg, bg)  # [N, E] sparse renormalized top-2 weights

    idxs, counts = [], []
    for e in range(E):
        idx = np.nonzero(w[:, e])[0]
        idxs.append(idx)
        counts.append(len(idx))
    # Device capacity: a multiple of 512 (uniform full-width pieces) sized to
    # the mean expert load; the few tokens past capacity (imbalance overflow,
    # ~1% of assignments) are computed on host in fp32 during the combine.
    mean_cap = 512 * max(1, int(round(N * KTOP / E / 512)))
    need_cap = 512 * (-(-max(counts) // 512))
    C = min(need_cap, mean_cap)
    pieces = _pieces(C)

    DSPLIT = BK * P  # first 768 dims bf16, last 256 dims fp8
    x_bf = x[:, :DSPLIT].astype(ml_dtypes.bfloat16)
    x_f8 = x[:, DSPLIT:].astype(ml_dtypes.float8_e4m3)
    in_maps = []
    for e in range(E):
        idx = idxs[e][:C]
        pad = np.zeros(C - len(idx), dtype=idx.dtype)
        idx_p = np.concatenate([idx, pad])
        xg = x_bf[idx_p]  # [C, 768] bf16
        xg8f = x_f8[idx_p]  # [C, 256] fp8
        # packing in processing order: piece -> [P, BK|FK, sz] -> concat
        chunks, chunks8 = [], []
        for pc in pieces:
            s0, sz = pc["s0"], pc["sz"]
            xs = xg[s0 : s0 + sz]  # [sz, 768]
            chunks.append(
                np.ascontiguousarray(
                    xs.T.reshape(BK, P, sz).transpose(1, 0, 2)
                ).reshape(P, BK * sz)
            )
            xs8 = xg8f[s0 : s0 + sz]  # [sz, 256]
            chunks8.append(
                np.ascontiguousarray(
                    xs8.T.reshape(FK, P, sz).transpose(1, 0, 2)
                ).reshape(P, FK * sz)
            )
        xgT = np.ascontiguousarray(np.concatenate(chunks, axis=1))
        xg8T = np.ascontiguousarray(np.concatenate(chunks8, axis=1))
        # w1 tiles (64x scale): [ht, p, k, m] = 64*W1[k*128+p, ht*128+m]
        w1_pm = np.ascontiguousarray(
            (W1[e][:DSPLIT] * np.float32(SCALE))
            .astype(ml_dtypes.bfloat16)
            .reshape(BK, P, HT, P)
            .transpose(2, 1, 0, 3)
        )
        w18_pm = np.ascontiguousarray(
            (W1[e][DSPLIT:] * np.float32(SCALE))
            .astype(ml_dtypes.float8_e4m3)
            .reshape(FK, P, HT, P)
            .transpose(2, 1, 0, 3)
        )
        # w2 moving (1/64 scale undoes mm1 scaling): [p, k, o] = W2[k*128+p, o]/64
        w2_pm = np.ascontiguousarray(
            (W2[e] * np.float32(1.0 / SCALE))
            .astype(ml_dtypes.bfloat16)
            .reshape(HK, P, O)
            .transpose(1, 0, 2)
        )
        b1_pm = np.ascontiguousarray(
            b1[e].reshape(HT, P).T * np.float32(SCALE)
        )
        in_maps.append(
            {"xgT": xgT, "xg8": xg8T, "w1": w1_pm, "w18": w18_pm, "w2": w2_pm, "b1": b1_pm}
        )

    nc = _build_program(C)
    res = None
    last_exc = None
    for attempt in range(3):
        try:
            res = run_bass_kernel_spmd(nc, in_maps, core_ids=list(range(NCORES)))
            break
        except Exception as exc:  # device wedge under profiling is transient
            last_exc = exc
            try:
                import jax

                jax.clear_caches()
            except Exception:
                pass
    if res is None:
        raise last_exc
    LAST_RESULTS = res

    out = np.zeros((N, O), dtype=np.float32)
    for e in range(E):
        c_dev = min(counts[e], C)
        idx_dev = idxs[e][:c_dev]
        yT = res.results[e]["yT"]  # [TT, P, O]
        y = yT.reshape(-1, O)[:c_dev]  # [c_dev, O]
        out[idx_dev] += w[idx_dev, e][:, None] * (y + b2[e])
        if counts[e] > C:  # capacity overflow: host fp32 fallback
            oidx = idxs[e][C:]
            yo = np.maximum(x[oidx] @ W1[e] + b1[e], 0.0) @ W2[e] + b2[e]
            out[oidx] += w[oidx, e][:, None] * yo
    return out


# revision 35
# speedup vs baseline: 1.0135x; 1.0135x over previous
"""Trainium2 Bass kernel for BoltzmannMoE (top-2 of 8 experts, N=8192, D=1024, H=4096, O=1024).

Strategy (expert-parallel across 8 NeuronCores):
  - Host: gate (softmax -> top-2 -> renormalize) in numpy fp32, gather each
    expert's tokens, run one expert per core, weighted scatter-add on host.
    Device capacity is the mean expert load (2048 = 4x512 uniform pieces);
    the ~1% of assignments past capacity fall back to host fp32.
  - Device (per core, SPMD), fp32 PSUM accumulation throughout; tokens
    processed in 512-column pieces. Per piece:
      mm1: h = relu(W1^T @ xg + b1)   (W1 tiles stationary, streamed per
           piece). 6 of 8 k-subtiles run in bf16; the last 2 run as a single
           fp8-e4m3 DoubleRow matmul (256-deep contraction per column
           stream, 2x rate). Everything accumulates at 64x scale -- W1*64
           fits fp8 range, relu commutes with the scale, and W2/64 undoes it,
           all exact power-of-2 folds. Measured end-to-end rel err 1.85e-2
           (gate 2e-2), bit-reproducible across runs.
      mm2: y[tok_tile] = ht^T @ W2    (token tiles stationary, W2 moving --
                                       W2 SBUF-resident, loaded once), bf16.
    A burst of junk matmuls on iota data at launch releases the PE HAM
    clock-gate (starts throttled ~2x) while the input DMAs spin up.
"""

import numpy as np
import ml_dtypes

import concourse.bass as bass
import concourse.mybir as mybir
import concourse.tile as tile
from concourse import bacc
from concourse.bass_utils import run_bass_kernel_spmd

P = 128
D, H, O, E, KTOP = 1024, 4096, 1024, 8, 2
TEMP = 2.718281828459045
NCORES = 8

DK = D // P    # 8  k-subtiles for mm1
BK = 6         # mm1 k-subtiles computed in bf16
FK = DK - BK   # 2  mm1 k-subtiles computed in fp8-e4m3 via one DoubleRow matmul
SCALE = 64.0   # power-of-2 fold: W1*64 (fp8/bf16 range), W2/64 undoes it
HK = H // P    # 32 k-subtiles for mm2
HT = H // P    # 32 h output tiles (mm1 M dim)
OHALF = 2      # mm2 output halves (512 cols each)
W1_POOL = 6    # w1 pool depth
W1_PRE = 2     # w1 tiles preloaded ahead of the first xg piece

BF16 = mybir.dt.bfloat16
F8 = mybir.dt.float8e4

LAST_RESULTS = None  # BassKernelResults of the most recent device run (for test harness)


def _pieces(C):
    """C is always a multiple of 512: uniform full-width pieces. Tokens
    beyond the device capacity are handled on the host (capacity-factor
    overflow), so no short-tail code path exists on device."""
    assert C % 512 == 0
    return [{"pack": i * 512, "s0": i * 512, "sz": 512} for i in range(C // 512)]


def _ttiles(sz):
    """Token-tiles (stationary blocks) within a piece."""
    out = []
    off = 0
    while off < sz:
        t = min(P, sz - off)
        out.append((off, t))
        off += t
    return out


def _build_program(C):
    nc = bacc.Bacc("TRN2", target_bir_lowering=False, debug=False)

    pieces = _pieces(C)
    TT = C // P  # token-tiles

    # xgT packed piece-major: piece p occupies flat cols
    # [BK*pack, BK*(pack+sz)) laid out as [BK, sz] (k-major within piece);
    # xg8 holds the last FK k-subtiles in fp8 as DoubleRow pair slots.
    xgT = nc.dram_tensor("xgT", (P, BK * C), BF16, kind="ExternalInput")
    xg8 = nc.dram_tensor("xg8", (P, FK * C), F8, kind="ExternalInput")
    w1 = nc.dram_tensor("w1", (HT, P, BK, P), BF16, kind="ExternalInput")
    w18 = nc.dram_tensor("w18", (HT, P, FK, P), F8, kind="ExternalInput")
    w2 = nc.dram_tensor("w2", (P, HK, O), BF16, kind="ExternalInput")
    b1 = nc.dram_tensor("b1", (P, HT), mybir.dt.float32, kind="ExternalInput")
    yT = nc.dram_tensor("yT", (TT, P, O), mybir.dt.float32, kind="ExternalOutput")

    with tile.TileContext(nc) as tc:
        with (
            tc.tile_pool(name="const", bufs=2) as const,
            tc.tile_pool(name="w1p", bufs=W1_POOL) as w1_pool,
            tc.tile_pool(name="w18p", bufs=W1_POOL) as w18_pool,
            tc.tile_pool(name="w2r", bufs=1) as w2_pool,
            tc.tile_pool(name="xg", bufs=3) as xg_pool,
            tc.tile_pool(name="xg8", bufs=3) as xg8_pool,
            tc.tile_pool(name="ht", bufs=2) as ht_pool,
            tc.tile_pool(name="yst", bufs=4) as yst_pool,
            tc.tile_pool(name="psj", bufs=1, space="PSUM") as psj,
            tc.tile_pool(name="psa", bufs=3, space="PSUM") as psa,
            tc.tile_pool(name="psb", bufs=4, space="PSUM") as psb,
        ):
            b1_sb = const.tile([P, HT], mybir.dt.float32)
            nc.sync.dma_start(b1_sb[:], b1.ap())

            def dma_xg_piece(pi):
                pack, sz = pieces[pi]["pack"], pieces[pi]["sz"]
                t = xg_pool.tile([P, BK, 512], BF16, name="xg_t")
                for k in range(BK):
                    nc.sync.dma_start(
                        t[:, k, :sz],
                        xgT.ap()[:, BK * pack + k * sz : BK * pack + (k + 1) * sz],
                    )
                t8 = xg8_pool.tile([P, FK, 512], F8, name="xg8_t")
                for j in range(FK):
                    nc.sync.dma_start(
                        t8[:, j, :sz],
                        xg8.ap()[:, FK * pack + j * sz : FK * pack + (j + 1) * sz],
                    )
                return t, t8

            # HAM warmup: the PE clock-gate starts throttled and releases
            # after ~4us of sustained activity (it watches data switching, so
            # the tile must hold varying values -- zeros don't warm it). Burn
            # junk matmuls on an iota tile (no DMA dependency, starts
            # immediately) while the input DMAs spin up, so real matmuls run
            # at full clock.
            junk = const.tile([P, P], BF16)
            nc.gpsimd.iota(
                junk[:],
                pattern=[[1, P]],
                base=1,
                channel_multiplier=7,
                allow_small_or_imprecise_dtypes=True,
            )
            ps_junk = psj.tile([P, 512], mybir.dt.float32, name="ps_junk")
            NJUNK = 40
            for j in range(NJUNK):
                nc.tensor.matmul(
                    ps_junk[:, :P],
                    junk[:],
                    junk[:],
                    start=(j == 0),
                    stop=(j == NJUNK - 1),
                )

            # piece-0 critical path: w1 tile 0, first xg k-slice, w1 tile 1,
            # rest of xg piece 0
            w1_pre, w18_pre = [], []

            def dma_w1(ht):
                t = w1_pool.tile([P, BK, P], BF16, name="w1_t")
                nc.sync.dma_start(t[:], w1.ap()[ht])
                t8 = w18_pool.tile([P, FK, P], F8, name="w18_t")
                nc.sync.dma_start(t8[:], w18.ap()[ht])
                return t, t8

            t, t8 = dma_w1(0)
            w1_pre.append(t)
            w18_pre.append(t8)
            pk0 = pieces[0]["pack"]
            xg_next = xg_pool.tile([P, BK, 512], BF16, name="xg_t")
            nc.sync.dma_start(xg_next[:, 0, :], xgT.ap()[:, BK * pk0 : BK * pk0 + 512])
            t, t8 = dma_w1(1)
            w1_pre.append(t)
            w18_pre.append(t8)
            for k in range(1, BK):
                nc.sync.dma_start(
                    xg_next[:, k, :],
                    xgT.ap()[:, BK * pk0 + k * 512 : BK * pk0 + (k + 1) * 512],
                )
            xg8_next = xg8_pool.tile([P, FK, 512], F8, name="xg8_t")
            for j in range(FK):
                nc.sync.dma_start(
                    xg8_next[:, j, :],
                    xg8.ap()[:, FK * pk0 + j * 512 : FK * pk0 + (j + 1) * 512],
                )

            w2_sb = w2_pool.tile([P, HK, O], BF16, name="w2_sb")

            for pi, pc in enumerate(pieces):
                s0, sz = pc["s0"], pc["sz"]
                xg_t, xg8_t = xg_next, xg8_next

                # ---- mm1: ht = relu(W1^T @ xg + b1), 64x-scaled psum ----
                ht_t = ht_pool.tile([P, HK, 512], BF16, name="ht_t")
                for ht in range(HT):
                    if pi == 0 and ht < W1_PRE:
                        w1_t, w18_t = w1_pre[ht], w18_pre[ht]
                    else:
                        w1_t, w18_t = dma_w1(ht)
                    ps = psa.tile([P, 512], mybir.dt.float32, name="ps_a")
                    for k in range(BK):
                        nc.tensor.matmul(
                            ps[:, :sz],
                            w1_t[:, k, :],
                            xg_t[:, k, :sz],
                            start=(k == 0),
                            stop=False,
                        )
                    # fp8 DoubleRow: both remaining k-subtiles in one matmul
                    nc.tensor.matmul(
                        ps[:, :sz],
                        w18_t[:],
                        xg8_t[:, :, :sz],
                        start=False,
                        stop=True,
                        perf_mode=mybir.MatmulPerfMode.DoubleRow,
                    )
                    nc.scalar.activation(
                        ht_t[:, ht, :sz],
                        ps[:, :sz],
                        mybir.ActivationFunctionType.Relu,
                        bias=b1_sb[:, ht : ht + 1],
                    )

                # next xg piece, then (once) the resident W2
                if pi + 1 < len(pieces):
                    xg_next, xg8_next = dma_xg_piece(pi + 1)
                if pi == 0:
                    for k in range(HK):
                        nc.sync.dma_start(w2_sb[:, k], w2.ap()[:, k])

                # ---- mm2: y[tok_tile] = ht^T @ W2 (tokens stationary) ----
                tt_base = s0 // P
                for ti, (toff, tw) in enumerate(_ttiles(sz)):
                    for oh in range(OHALF):
                        ps = psb.tile([P, 512], mybir.dt.float32, name="ps_b")
                        for k in range(HK):
                            nc.tensor.matmul(
                                ps[:tw, :],
                                ht_t[:, k, toff : toff + tw],
                                w2_sb[:, k, oh * 512 : (oh + 1) * 512],
                                start=(k == 0),
                                stop=(k == HK - 1),
                            )
                        st = yst_pool.tile([P, 512], mybir.dt.float32, name="y_st")
                        nc.vector.tensor_copy(st[:tw, :], ps[:tw, :])
                        nc.sync.dma_start(
                            yT.ap()[tt_base + ti][:tw, oh * 512 : (oh + 1) * 512],
                            st[:tw, :],
                        )

    nc.compile()
    return nc


def _host_gate(x, Wg, bg):
    """Replicates reference gating in fp32: softmax(scores/T) -> top-2 -> renorm."""
    scores = (x @ Wg + bg) / np.float32(TEMP)
    m = scores.max(axis=-1, keepdims=True)
    un = np.exp(scores - m)
    probs = un / un.sum(-1, keepdims=True)
    order = np.argsort(-probs, axis=1, kind="stable")[:, :KTOP]
    vals = np.take_along_axis(probs, order, axis=1)
    w = np.zeros_like(probs)
    np.put_along_axis(w, order, vals, axis=1)
    w = w / (w.sum(-1, keepdims=True) + np.float32(1e-8))
    return w


def kernel(x, Wg, bg, W1, b1, W2, b2):
    global LAST_RESULTS
    x = np.ascontiguousarray(np.asarray(x, dtype=np.float32))
    Wg = np.asarray(Wg, dtype=np.float32)
    bg = np.asarray(bg, dtype=np.float32)
    W1 = np.asarray(W1, dtype=np.float32)
    b1 = np.asarray(b1, dtype=np.float32)
    W2 = np.asarray(W2, dtype=np.float32)
    b2 = np.asarray(b2, dtype=np.float32)
    N = x.shape[0]

    w = _host_gate(x, Wg, bg)  # [N, E] sparse renormalized top-2 weights

    idxs, counts = [], []
    for e in range(E):
        idx = np.nonzero(w[:, e])[0]
        idxs.append(idx)
        counts.append(len(idx))
    # Device capacity: a multiple of 512 (uniform full-width pieces) sized to
    # the mean expert load; the few tokens past capacity (imbalance overflow,
    # ~1% of assignments) are computed on host in fp32 during the combine.
    mean_cap = 512 * max(1, int(round(N * KTOP / E / 512)))
    need_cap = 512 * (-(-max(counts) // 512))
    C = min(need_cap, mean_cap)
    pieces = _pieces(C)

    DSPLIT = BK * P  # first 768 dims bf16, last 256 dims fp8
    x_bf = x[:, :DSPLIT].astype(ml_dtypes.bfloat16)
    x_f8 = x[:, DSPLIT:].astype(ml_dtypes.float8_e4m3)
    in_maps = []
    for e in range(E):
        idx = idxs[e][:C]
        pad = np.zeros(C - len(idx), dtype=idx.dtype)
        idx_p = np.concatenate([idx, pad])
        xg = x_bf[idx_p]  # [C, 768] bf16
        xg8f = x_f8[idx_p]  # [C, 256] fp8
        # packing in processing order: piece -> [P, BK|FK, sz] -> concat
        chunks, chunks8 = [], []
        for pc in pieces:
            s0, sz = pc["s0"], pc["sz"]
            xs = xg[s0 : s0 + sz]  # [sz, 768]
            chunks.append(
                np.ascontiguousarray(
                    xs.T.reshape(BK, P, sz).transpose(1, 0, 2)
                ).reshape(P, BK * sz)
            )
            xs8 = xg8f[s0 : s0 + sz]  # [sz, 256]
            chunks8.append(
                np.ascontiguousarray(
                    xs8.T.reshape(FK, P, sz).transpose(1, 0, 2)
                ).reshape(P, FK * sz)
            )
        xgT = np.ascontiguousarray(np.concatenate(chunks, axis=1))
        xg8T = np.ascontiguousarray(np.concatenate(chunks8, axis=1))
        # w1 tiles (64x scale): [ht, p, k, m] = 64*W1[k*128+p, ht*128+m]
        w1_pm = np.ascontiguousarray(
            (W1[e][:DSPLIT] * np.float32(SCALE))
            .astype(ml_dtypes.bfloat16)
            .reshape(BK, P, HT, P)
            .transpose(2, 1, 0, 3)
        )
        w18_pm = np.ascontiguousarray(
            (W1[e][DSPLIT:] * np.float32(SCALE))
            .astype(ml_dtypes.float8_e4m3)
            .reshape(FK, P, HT, P)
            .transpose(2, 1, 0, 3)
        )
        # w2 moving (1/64 scale undoes mm1 scaling): [p, k, o] = W2[k*128+p, o]/64
        w2_pm = np.ascontiguousarray(
            (W2[e] * np.float32(1.0 / SCALE))
            .astype(ml_dtypes.bfloat16)
            .reshape(HK, P, O)
            .transpose(1, 0, 2)
        )
        b1_pm = np.ascontiguousarray(
            b1[e].reshape(HT, P).T * np.float32(SCALE)
        )
        in_maps.append(
            {"xgT": xgT, "xg8": xg8T, "w1": w1_pm, "w18": w18_pm, "w2": w2_pm, "b1": b1_pm}
        )

    nc = _build_program(C)
    res = None
    last_exc = None
    for attempt in range(4):
        try:
            res = run_bass_kernel_spmd(nc, in_maps, core_ids=list(range(NCORES)))
            break
        except Exception as exc:  # device wedge under profiling is transient
            last_exc = exc
            try:
                import jax

                jax.clear_caches()
            except Exception:
                pass
            import time as _time

            _time.sleep(5 * (attempt + 1))
    if res is None:
        raise last_exc
    LAST_RESULTS = res

    out = np.zeros((N, O), dtype=np.float32)
    for e in range(E):
        c_dev = min(counts[e], C)
        idx_dev = idxs[e][:c_dev]
        yT = res.results[e]["yT"]  # [TT, P, O]
        y = yT.reshape(-1, O)[:c_dev]  # [c_dev, O]
        out[idx_dev] += w[idx_dev, e][:, None] * (y + b2[e])
        if counts[e] > C:  # capacity overflow: host fp32 fallback
            oidx = idxs[e][C:]
            yo = np.maximum(x[oidx] @ W1[e] + b1[e], 0.0) @ W2[e] + b2[e]
            out[oidx] += w[oidx, e][:, None] * yo
    return out
